# revision 10
# baseline (speedup 1.0000x reference)
"""Cross-attention kernel for Trainium2 (8 NeuronCores, SPMD data-parallel).

Problem: O = softmax(Q @ K^T) @ V with B=4, Lq=Lk=4096, D=64, fp32 (no
1/sqrt(d) scaling).

Sharding: 8 cores = 4 batches x 2 Lq-halves. Each core handles a
[2048, 64] Q shard against the full [4096, 64] K/V of its batch.
Independent outputs -> no collectives.

Per-core algorithm (HW model: the PE output bus serializes matmuls at 128
results/cycle, so row-tiled pairing gains nothing; the PE clock ramps
0.65 -> 1.2 -> 2.4 GHz only while the matmul stream is continuously busy.
The kernel is built so the PE never waits):
  - ST[k, q] = matmul(lhsT=KT chunk [64,128], rhs=QT [64,512]) -> PSUM
    [128, 1024] tiles, 3 PSUM tiles deep so scores run 3 chunks ahead.
  - exp alternates engines so neither is the bottleneck:
      even chunks: scalar ACTIVATE exact exp -> bf16
      odd  chunks: DVE Schraudolph bit-trick exp: round(x*2^7/ln2 +
      (127*128 - 7.7)) as int16, bitcast bf16 (~3% systematic rel err on
      those chunks; final output rel-L2 ~5.4e-3, tol 2e-2).
  - OT[65, q] += matmul(lhsT=VA chunk [128, 65] bf16, rhs=PT [128, 512]):
    VA = concat([V, ones], 1); rows 0..63 accumulate unnormalized output,
    row 64 the softmax denominator (exp source consistent per chunk).
  - Normalize: OT -> SBUF copy (frees PSUM fast for the next q-block),
    fast-reciprocal of row 64, gpsimd partition-broadcast, multiply, DMA
    out OT [64, 2048] f32; host transposes back.
"""

import sys

for _p in ("/opt/trn_rl_repo", "/opt/pypackages"):
    if _p not in sys.path:
        sys.path.insert(0, _p)

from contextlib import ExitStack

import ml_dtypes
import numpy as np

import concourse.bacc as bacc
import concourse.mybir as mybir
import concourse.tile as tile
from concourse.bass_utils import run_bass_kernel_spmd

# Problem constants (hardcoded per contract).
B, LQ, LK, D = 4, 4096, 4096, 64
N_CORES = 8
LQ_SHARD = LQ * B // N_CORES  # 2048
QB = 1024  # q-block (exp instruction free-size; 2 PSUM banks)
NQB = LQ_SHARD // QB  # 2
KC = 128  # k-chunk (contraction tile for the PV matmul)
NKC = LK // KC  # 32
SL = 512  # matmul moving-dim slice (one PSUM bank)
NSL = QB // SL  # 2

F32 = mybir.dt.float32
F16 = mybir.dt.float16
BF16 = mybir.dt.bfloat16
I16 = mybir.dt.int16

BF16NP = ml_dtypes.bfloat16

# Schraudolph exp -> bf16 bit pattern: round(x * 2^7/ln2 + 127*128 - C).
EXP_A = 128.0 / float(np.log(2.0))
EXP_C = 7.7
EXP_B = 127.0 * 128.0 - EXP_C

KT_PIECE = 512  # kt DMA piece width (4 k-chunks)
VA_PIECE = 8  # va DMA piece size in k-chunks
ST_BUFS = 3
PT_BUFS = 4


def _build_program():
    nc = bacc.Bacc(
        "TRN2",
        target_bir_lowering=False,
        debug=False,
        num_devices=N_CORES,
    )
    qt_d = nc.declare_dram_parameter("QT", [D, LQ_SHARD], F16, isOutput=False)
    kt_d = nc.declare_dram_parameter("KT", [D, LK], F16, isOutput=False)
    va_d = nc.declare_dram_parameter("VA", [LK, D + 1], BF16, isOutput=False)
    ot_d = nc.declare_dram_parameter("OT", [D, LQ_SHARD], F32, isOutput=True)

    with tile.TileContext(nc) as tc, ExitStack() as ctx:
        singles = ctx.enter_context(tc.tile_pool(name="singles", bufs=1))
        st_pool = ctx.enter_context(
            tc.tile_pool(name="st", bufs=ST_BUFS, space="PSUM")
        )
        ot_pool = ctx.enter_context(tc.tile_pool(name="ot", bufs=1, space="PSUM"))
        pt_pool = ctx.enter_context(tc.tile_pool(name="pt", bufs=PT_BUFS))
        osb_pool = ctx.enter_context(tc.tile_pool(name="osb", bufs=2))
        out_pool = ctx.enter_context(tc.tile_pool(name="out", bufs=4))
        norm_pool = ctx.enter_context(tc.tile_pool(name="norm", bufs=8))

        # Preload the exp activation table while input DMAs run.
        warm = singles.tile([1, 2], F32)
        nc.vector.memset(warm[:, :], 0.0)
        nc.scalar.activation(
            out=warm[:, :], in_=warm[:, :],
            func=mybir.ActivationFunctionType.Exp,
        )

        # PE clock boost trigger: the p-state governor raises the PE clock
        # (1.2 -> 2.4 GHz) only once high demand is established AND the PE
        # goes idle for ~2us (the frequency switch applies during an idle
        # window; observed at the q-block transition, after which matmuls
        # run at 2.4 GHz). Engineer that idle window early: one dummy
        # matmul a few chunks into the first q-block, gated on a serial
        # gpsimd delay chain timed to stall the PE ~2.5us.
        wu_w = singles.tile([D, 16], F16, name="wu_w")
        nc.vector.memset(wu_w[:, :], 0.0)
        delay_tiles = []
        for i in range(4):
            delay_tiles.append(
                singles.tile([KC, 2400], F32, name=f"wu_delay{i}")
            )
        nc.gpsimd.memset(delay_tiles[0][:, :], 0.0)
        for i in range(3):
            nc.gpsimd.tensor_copy(
                delay_tiles[i + 1][:, :], delay_tiles[i][:, :]
            )

        qt_sb = singles.tile([D, LQ_SHARD], F16, name="qt")
        kt_sb = singles.tile([D, LK], F16, name="kt")
        va_sb = singles.tile([KC, NKC, D + 1], BF16, name="va")
        va_r = va_d[:, :].rearrange("(c p) d -> p c d", p=KC)

        # Input DMAs, issued in consumption order (kt chunks gate the
        # score matmuls; va chunks the PV matmuls; qt half 1 only needed
        # at the second q-block).
        def dma_kt(j):
            sl = slice(j * KT_PIECE, (j + 1) * KT_PIECE)
            nc.sync.dma_start(out=kt_sb[:, sl], in_=kt_d[:, sl])

        def dma_qt(h):
            sl = slice(h * QB, (h + 1) * QB)
            nc.sync.dma_start(out=qt_sb[:, sl], in_=qt_d[:, sl])

        def dma_va(h):
            sl = slice(h * VA_PIECE, (h + 1) * VA_PIECE)
            nc.sync.dma_start(out=va_sb[:, sl, :], in_=va_r[:, sl, :])

        dma_kt(0)
        dma_qt(0)
        dma_va(0)
        dma_kt(1)
        dma_kt(2)
        dma_va(1)
        dma_kt(3)
        dma_kt(4)
        dma_qt(1)
        dma_va(2)
        dma_kt(5)
        dma_kt(6)
        dma_va(3)
        dma_kt(7)

        def emit_scores(qb, c, st_ps):
            for s in range(NSL):
                sl = slice(s * SL, (s + 1) * SL)
                qsl = slice(qb * QB + s * SL, qb * QB + (s + 1) * SL)
                nc.tensor.matmul(
                    out=st_ps[:, sl],
                    lhsT=kt_sb[:, c * KC : (c + 1) * KC],
                    rhs=qt_sb[:, qsl],
                    start=True,
                    stop=True,
                )

        def emit_exp(c, st_ps, pt):
            # pt is an int16 tile; both writers produce bf16 bit patterns.
            if c % 2 == 0:
                nc.scalar.activation(
                    out=pt[:, :].bitcast(BF16),
                    in_=st_ps[:, :],
                    func=mybir.ActivationFunctionType.Exp,
                )
            else:
                nc.vector.tensor_scalar(
                    out=pt[:, :],
                    in0=st_ps[:, :],
                    scalar1=EXP_A,
                    scalar2=EXP_B,
                    op0=mybir.AluOpType.mult,
                    op1=mybir.AluOpType.add,
                )

        def emit_pv(c, pt, ot_ps):
            for s in range(NSL):
                sl = slice(s * SL, (s + 1) * SL)
                nc.tensor.matmul(
                    out=ot_ps[:, sl],
                    lhsT=va_sb[:, c, :],
                    rhs=pt[:, sl].bitcast(BF16),
                    start=(c == 0),
                    stop=(c == NKC - 1),
                )

        for qb in range(NQB):
            ot_ps = ot_pool.tile([D + 1, QB], F32)
            st_tiles = {}
            for c in range(ST_BUFS):  # prologue: scores run ahead
                st_tiles[c] = st_pool.tile([KC, QB], F32, tag="st", name="st")
                emit_scores(qb, c, st_tiles[c])
            for c in range(NKC):
                if c + ST_BUFS < NKC:
                    st_tiles[c + ST_BUFS] = st_pool.tile(
                        [KC, QB], F32, tag="st", name="st"
                    )
                    if qb == 0 and c == 3:
                        # Dummy matmul gated on the delay chain: stalls the
                        # PE ~2.5us so the clock boost can apply. Its
                        # garbage output is overwritten by the start=True
                        # score matmuls that follow.
                        nc.tensor.matmul(
                            out=st_tiles[c + ST_BUFS][0:16, 0:SL],
                            lhsT=wu_w[:, :],
                            rhs=delay_tiles[3][0:D, 0 : SL // 2].bitcast(
                                F16
                            ),
                            start=True,
                            stop=True,
                        )
                    emit_scores(qb, c + ST_BUFS, st_tiles[c + ST_BUFS])
                pt = pt_pool.tile([KC, QB], I16, tag="pt", name="pt")
                emit_exp(c, st_tiles.pop(c), pt)
                emit_pv(c, pt, ot_ps)

            # Normalize O[d, q] = OT[d, q] / OT[64, q], in halves so the
            # recip/broadcast/mul/DMA chain pipelines across engines.
            if qb < NQB - 1:
                # Free the OT PSUM banks quickly for the next q-block.
                osb = osb_pool.tile([D + 1, QB], F32)
                nc.vector.tensor_copy(osb[:, :], ot_ps[:, :])
                src = osb
            else:
                src = ot_ps
            for h in range(2):
                sl = slice(h * SL, (h + 1) * SL)
                den = norm_pool.tile([1, SL], F32)
                nc.vector.tensor_copy(den[:, :], src[D : D + 1, sl])
                recip = norm_pool.tile([1, SL], F32)
                nc.vector.reciprocal_approx_fast(recip[:, :], den[:, :])
                bcast = norm_pool.tile([D, SL], F32)
                nc.gpsimd.partition_broadcast(bcast[:, :], recip[:, :])
                o_sb = out_pool.tile([D, SL], F32)
                nc.vector.tensor_mul(o_sb[:, :], src[0:D, sl], bcast[:, :])
                osl = slice(qb * QB + h * SL, qb * QB + (h + 1) * SL)
                nc.sync.dma_start(out=ot_d[:, osl], in_=o_sb[:, :])

    nc.finalize()
    return nc


_PROGRAM_CACHE = {}


def _get_program():
    if "nc" not in _PROGRAM_CACHE:
        _PROGRAM_CACHE["nc"] = _build_program()
    return _PROGRAM_CACHE["nc"]


def _make_in_maps(Q, K, V):
    Q = np.asarray(Q, dtype=np.float32)
    K = np.asarray(K, dtype=np.float32)
    V = np.asarray(V, dtype=np.float32)
    in_maps = []
    ones = np.ones((LK, 1), dtype=np.float32)
    for core in range(N_CORES):
        b, half = core // 2, core % 2
        q_shard = Q[b, half * LQ_SHARD : (half + 1) * LQ_SHARD, :]  # [2048, 64]
        qt = np.ascontiguousarray(q_shard.T).astype(np.float16)  # [64, 2048]
        kt = np.ascontiguousarray(K[b].T).astype(np.float16)  # [64, 4096]
        va = np.concatenate([V[b], ones], axis=1).astype(BF16NP)  # [4096, 65]
        in_maps.append({"QT": qt, "KT": kt, "VA": np.ascontiguousarray(va)})
    return in_maps


def _run(Q, K, V, trace=False, **spmd_kwargs):
    nc = _get_program()
    in_maps = _make_in_maps(Q, K, V)
    res = run_bass_kernel_spmd(
        nc, in_maps, list(range(N_CORES)), trace=trace, **spmd_kwargs
    )
    out = np.empty((B, LQ, D), dtype=np.float32)
    for core in range(N_CORES):
        b, half = core // 2, core % 2
        ot = res.results[core]["OT"]  # [64, 2048]
        out[b, half * LQ_SHARD : (half + 1) * LQ_SHARD, :] = ot.T
    return out, res


def kernel(Q, K, V):
    out, _ = _run(Q, K, V, trace=False)
    return out


# revision 12
# speedup vs baseline: 1.0980x; 1.0980x over previous
"""Cross-attention kernel for Trainium2 (8 NeuronCores, SPMD data-parallel).

Problem: O = softmax(Q @ K^T) @ V with B=4, Lq=Lk=4096, D=64, fp32 (no
1/sqrt(d) scaling).

Sharding: 8 cores = 4 batches x 2 Lq-halves. Each core handles a
[2048, 64] Q shard against the full [4096, 64] K/V of its batch.
Independent outputs -> no collectives.

Per-core algorithm (HW model: the PE output bus serializes matmuls at 128
results/cycle, so row-tiled pairing gains nothing; the PE clock ramps
0.65 -> 1.2 -> 2.4 GHz only while the matmul stream is continuously busy.
The kernel is built so the PE never waits):
  - ST[k, q] = matmul(lhsT=KT chunk [64,128], rhs=QT [64,512]) -> PSUM
    [128, 1024] tiles, 3 PSUM tiles deep so scores run 3 chunks ahead.
  - exp alternates engines so neither is the bottleneck:
      even chunks: scalar ACTIVATE exact exp -> bf16
      odd  chunks: DVE Schraudolph bit-trick exp: round(x*2^7/ln2 +
      (127*128 - 7.7)) as int16, bitcast bf16 (~3% systematic rel err on
      those chunks; final output rel-L2 ~5.4e-3, tol 2e-2).
  - OT[65, q] += matmul(lhsT=VA chunk [128, 65] bf16, rhs=PT [128, 512]):
    VA = concat([V, ones], 1); rows 0..63 accumulate unnormalized output,
    row 64 the softmax denominator (exp source consistent per chunk).
  - Normalize: OT -> SBUF copy (frees PSUM fast for the next q-block),
    fast-reciprocal of row 64, gpsimd partition-broadcast, multiply, DMA
    out OT [64, 2048] f32; host transposes back.
"""

import sys

for _p in ("/opt/trn_rl_repo", "/opt/pypackages"):
    if _p not in sys.path:
        sys.path.insert(0, _p)

from contextlib import ExitStack

import ml_dtypes
import numpy as np

import concourse.bacc as bacc
import concourse.mybir as mybir
import concourse.tile as tile
from concourse.bass_utils import run_bass_kernel_spmd

# Problem constants (hardcoded per contract).
B, LQ, LK, D = 4, 4096, 4096, 64
N_CORES = 8
LQ_SHARD = LQ * B // N_CORES  # 2048
QB = 1024  # q-block (exp instruction free-size; 2 PSUM banks)
NQB = LQ_SHARD // QB  # 2
KC = 128  # k-chunk (contraction tile for the PV matmul)
NKC = LK // KC  # 32
SL = 512  # matmul moving-dim slice (one PSUM bank)
NSL = QB // SL  # 2

F32 = mybir.dt.float32
F16 = mybir.dt.float16
BF16 = mybir.dt.bfloat16
I16 = mybir.dt.int16

BF16NP = ml_dtypes.bfloat16

# Schraudolph exp -> bf16 bit pattern: round(x * 2^7/ln2 + 127*128 - C).
EXP_A = 128.0 / float(np.log(2.0))
EXP_C = 7.7
EXP_B = 127.0 * 128.0 - EXP_C

KT_PIECE = 512  # kt DMA piece width (4 k-chunks)
VA_PIECE = 8  # va DMA piece size in k-chunks
ST_BUFS = 3
PT_BUFS = 4


def _build_program():
    nc = bacc.Bacc(
        "TRN2",
        target_bir_lowering=False,
        debug=False,
        num_devices=N_CORES,
    )
    qt_d = nc.declare_dram_parameter("QT", [D, LQ_SHARD], F16, isOutput=False)
    kt_d = nc.declare_dram_parameter("KT", [D, LK], F16, isOutput=False)
    va_d = nc.declare_dram_parameter("VA", [LK, D + 1], BF16, isOutput=False)
    ot_d = nc.declare_dram_parameter("OT", [D, LQ_SHARD], F32, isOutput=True)

    with tile.TileContext(nc) as tc, ExitStack() as ctx:
        singles = ctx.enter_context(tc.tile_pool(name="singles", bufs=1))
        st_pool = ctx.enter_context(
            tc.tile_pool(name="st", bufs=ST_BUFS, space="PSUM")
        )
        ot_pool = ctx.enter_context(tc.tile_pool(name="ot", bufs=1, space="PSUM"))
        pt_pool = ctx.enter_context(tc.tile_pool(name="pt", bufs=PT_BUFS))
        osb_pool = ctx.enter_context(tc.tile_pool(name="osb", bufs=2))
        out_pool = ctx.enter_context(tc.tile_pool(name="out", bufs=4))
        norm_pool = ctx.enter_context(tc.tile_pool(name="norm", bufs=8))

        # Preload the exp activation table while input DMAs run.
        warm = singles.tile([1, 2], F32)
        nc.vector.memset(warm[:, :], 0.0)
        nc.scalar.activation(
            out=warm[:, :], in_=warm[:, :],
            func=mybir.ActivationFunctionType.Exp,
        )

        # PE clock boost trigger: the p-state governor raises the PE clock
        # (1.2 -> 2.4 GHz) only once high demand is established AND the PE
        # goes idle for ~2us (the frequency switch applies during an idle
        # window; observed at the q-block transition, after which matmuls
        # run at 2.4 GHz). Engineer that idle window early: one dummy
        # matmul a few chunks into the first q-block, gated on a serial
        # gpsimd delay chain timed to stall the PE ~2.5us.
        # Measured gpsimd rates: memset [128,2400]f32 = 2.1us; tensor_copy
        # = 3.4ns/elem/partition. Chain = memset + copy(2900) ends ~21.4us
        # (gpsimd ready ~9us); the PE drains chunk 3 at ~18.5-19us, giving
        # a ~2.5us stall -- in the promote band, well below the ~10us+
        # demote band.
        wu_w = singles.tile([D, 16], F16, name="wu_w")
        nc.vector.memset(wu_w[:, :], 0.0)
        delay_tiles = []
        for i in range(2):
            delay_tiles.append(
                singles.tile([KC, 2900], F32, name=f"wu_delay{i}")
            )
        nc.gpsimd.memset(delay_tiles[0][:, :], 0.0)
        nc.gpsimd.tensor_copy(delay_tiles[1][:, :], delay_tiles[0][:, :])

        qt_sb = singles.tile([D, LQ_SHARD], F16, name="qt")
        kt_sb = singles.tile([D, LK], F16, name="kt")
        va_sb = singles.tile([KC, NKC, D + 1], BF16, name="va")
        va_r = va_d[:, :].rearrange("(c p) d -> p c d", p=KC)

        # Input DMAs, issued in consumption order (kt chunks gate the
        # score matmuls; va chunks the PV matmuls; qt half 1 only needed
        # at the second q-block).
        def dma_kt(j):
            sl = slice(j * KT_PIECE, (j + 1) * KT_PIECE)
            nc.sync.dma_start(out=kt_sb[:, sl], in_=kt_d[:, sl])

        def dma_qt(h):
            sl = slice(h * QB, (h + 1) * QB)
            nc.sync.dma_start(out=qt_sb[:, sl], in_=qt_d[:, sl])

        def dma_va(h):
            sl = slice(h * VA_PIECE, (h + 1) * VA_PIECE)
            nc.sync.dma_start(out=va_sb[:, sl, :], in_=va_r[:, sl, :])

        dma_kt(0)
        dma_qt(0)
        dma_va(0)
        dma_kt(1)
        dma_kt(2)
        dma_va(1)
        dma_kt(3)
        dma_kt(4)
        dma_qt(1)
        dma_va(2)
        dma_kt(5)
        dma_kt(6)
        dma_va(3)
        dma_kt(7)

        def emit_scores(qb, c, st_ps):
            for s in range(NSL):
                sl = slice(s * SL, (s + 1) * SL)
                qsl = slice(qb * QB + s * SL, qb * QB + (s + 1) * SL)
                nc.tensor.matmul(
                    out=st_ps[:, sl],
                    lhsT=kt_sb[:, c * KC : (c + 1) * KC],
                    rhs=qt_sb[:, qsl],
                    start=True,
                    stop=True,
                )

        def emit_exp(c, st_ps, pt):
            # pt is an int16 tile; both writers produce bf16 bit patterns.
            if c % 2 == 0:
                nc.scalar.activation(
                    out=pt[:, :].bitcast(BF16),
                    in_=st_ps[:, :],
                    func=mybir.ActivationFunctionType.Exp,
                )
            else:
                nc.vector.tensor_scalar(
                    out=pt[:, :],
                    in0=st_ps[:, :],
                    scalar1=EXP_A,
                    scalar2=EXP_B,
                    op0=mybir.AluOpType.mult,
                    op1=mybir.AluOpType.add,
                )

        def emit_pv(c, pt, ot_ps):
            for s in range(NSL):
                sl = slice(s * SL, (s + 1) * SL)
                nc.tensor.matmul(
                    out=ot_ps[:, sl],
                    lhsT=va_sb[:, c, :],
                    rhs=pt[:, sl].bitcast(BF16),
                    start=(c == 0),
                    stop=(c == NKC - 1),
                )

        for qb in range(NQB):
            ot_ps = ot_pool.tile([D + 1, QB], F32)
            st_tiles = {}
            for c in range(ST_BUFS):  # prologue: scores run ahead
                st_tiles[c] = st_pool.tile([KC, QB], F32, tag="st", name="st")
                emit_scores(qb, c, st_tiles[c])
            for c in range(NKC):
                if c + ST_BUFS < NKC:
                    st_tiles[c + ST_BUFS] = st_pool.tile(
                        [KC, QB], F32, tag="st", name="st"
                    )
                    if qb == 0 and c == 3:
                        # Dummy matmul gated on the delay chain: stalls the
                        # PE ~2.5us so the clock boost can apply. Its
                        # garbage output is overwritten by the start=True
                        # score matmuls that follow.
                        nc.tensor.matmul(
                            out=st_tiles[c + ST_BUFS][0:16, 0:SL],
                            lhsT=wu_w[:, :],
                            rhs=delay_tiles[1][0:D, 0 : SL // 2].bitcast(
                                F16
                            ),
                            start=True,
                            stop=True,
                        )
                    emit_scores(qb, c + ST_BUFS, st_tiles[c + ST_BUFS])
                pt = pt_pool.tile([KC, QB], I16, tag="pt", name="pt")
                emit_exp(c, st_tiles.pop(c), pt)
                emit_pv(c, pt, ot_ps)

            # Normalize O[d, q] = OT[d, q] / OT[64, q], in halves so the
            # recip/broadcast/mul/DMA chain pipelines across engines.
            if qb < NQB - 1:
                # Free the OT PSUM banks quickly for the next q-block.
                osb = osb_pool.tile([D + 1, QB], F32)
                nc.vector.tensor_copy(osb[:, :], ot_ps[:, :])
                src = osb
            else:
                src = ot_ps
            for h in range(2):
                sl = slice(h * SL, (h + 1) * SL)
                den = norm_pool.tile([1, SL], F32)
                nc.vector.tensor_copy(den[:, :], src[D : D + 1, sl])
                recip = norm_pool.tile([1, SL], F32)
                nc.vector.reciprocal_approx_fast(recip[:, :], den[:, :])
                bcast = norm_pool.tile([D, SL], F32)
                nc.gpsimd.partition_broadcast(bcast[:, :], recip[:, :])
                o_sb = out_pool.tile([D, SL], F32)
                nc.vector.tensor_mul(o_sb[:, :], src[0:D, sl], bcast[:, :])
                osl = slice(qb * QB + h * SL, qb * QB + (h + 1) * SL)
                nc.sync.dma_start(out=ot_d[:, osl], in_=o_sb[:, :])

    nc.finalize()
    return nc


_PROGRAM_CACHE = {}


def _get_program():
    if "nc" not in _PROGRAM_CACHE:
        _PROGRAM_CACHE["nc"] = _build_program()
    return _PROGRAM_CACHE["nc"]


def _make_in_maps(Q, K, V):
    Q = np.asarray(Q, dtype=np.float32)
    K = np.asarray(K, dtype=np.float32)
    V = np.asarray(V, dtype=np.float32)
    in_maps = []
    ones = np.ones((LK, 1), dtype=np.float32)
    for core in range(N_CORES):
        b, half = core // 2, core % 2
        q_shard = Q[b, half * LQ_SHARD : (half + 1) * LQ_SHARD, :]  # [2048, 64]
        qt = np.ascontiguousarray(q_shard.T).astype(np.float16)  # [64, 2048]
        kt = np.ascontiguousarray(K[b].T).astype(np.float16)  # [64, 4096]
        va = np.concatenate([V[b], ones], axis=1).astype(BF16NP)  # [4096, 65]
        in_maps.append({"QT": qt, "KT": kt, "VA": np.ascontiguousarray(va)})
    return in_maps


def _run(Q, K, V, trace=False, **spmd_kwargs):
    nc = _get_program()
    in_maps = _make_in_maps(Q, K, V)
    res = run_bass_kernel_spmd(
        nc, in_maps, list(range(N_CORES)), trace=trace, **spmd_kwargs
    )
    out = np.empty((B, LQ, D), dtype=np.float32)
    for core in range(N_CORES):
        b, half = core // 2, core % 2
        ot = res.results[core]["OT"]  # [64, 2048]
        out[b, half * LQ_SHARD : (half + 1) * LQ_SHARD, :] = ot.T
    return out, res


def kernel(Q, K, V):
    out, _ = _run(Q, K, V, trace=False)
    return out


# revision 14
# speedup vs baseline: 1.0991x; 1.0010x over previous
"""Cross-attention kernel for Trainium2 (8 NeuronCores, SPMD data-parallel).

Problem: O = softmax(Q @ K^T) @ V with B=4, Lq=Lk=4096, D=64, fp32 (no
1/sqrt(d) scaling).

Sharding: 8 cores = 4 batches x 2 Lq-halves. Each core handles a
[2048, 64] Q shard against the full [4096, 64] K/V of its batch.
Independent outputs -> no collectives.

Per-core algorithm (HW model: the PE output bus serializes matmuls at 128
results/cycle, so row-tiled pairing gains nothing; the PE clock ramps
0.65 -> 1.2 -> 2.4 GHz only while the matmul stream is continuously busy.
The kernel is built so the PE never waits):
  - ST[k, q] = matmul(lhsT=KT chunk [64,128], rhs=QT [64,512]) -> PSUM
    [128, 1024] tiles, 3 PSUM tiles deep so scores run 3 chunks ahead.
  - exp alternates engines so neither is the bottleneck:
      even chunks: scalar ACTIVATE exact exp -> bf16
      odd  chunks: DVE Schraudolph bit-trick exp: round(x*2^7/ln2 +
      (127*128 - 7.7)) as int16, bitcast bf16 (~3% systematic rel err on
      those chunks; final output rel-L2 ~5.4e-3, tol 2e-2).
  - OT[65, q] += matmul(lhsT=VA chunk [128, 65] bf16, rhs=PT [128, 512]):
    VA = concat([V, ones], 1); rows 0..63 accumulate unnormalized output,
    row 64 the softmax denominator (exp source consistent per chunk).
  - Normalize: OT -> SBUF copy (frees PSUM fast for the next q-block),
    fast-reciprocal of row 64, gpsimd partition-broadcast, multiply, DMA
    out OT [64, 2048] f32; host transposes back.
"""

import sys

for _p in ("/opt/trn_rl_repo", "/opt/pypackages"):
    if _p not in sys.path:
        sys.path.insert(0, _p)

from contextlib import ExitStack

import ml_dtypes
import numpy as np

import concourse.bacc as bacc
import concourse.mybir as mybir
import concourse.tile as tile
from concourse.bass_utils import run_bass_kernel_spmd

# Problem constants (hardcoded per contract).
B, LQ, LK, D = 4, 4096, 4096, 64
N_CORES = 8
LQ_SHARD = LQ * B // N_CORES  # 2048
QB = 1024  # q-block (exp instruction free-size; 2 PSUM banks)
NQB = LQ_SHARD // QB  # 2
KC = 128  # k-chunk (contraction tile for the PV matmul)
NKC = LK // KC  # 32
SL = 512  # matmul moving-dim slice (one PSUM bank)
NSL = QB // SL  # 2

F32 = mybir.dt.float32
F16 = mybir.dt.float16
BF16 = mybir.dt.bfloat16
I16 = mybir.dt.int16

BF16NP = ml_dtypes.bfloat16

# Schraudolph exp -> bf16 bit pattern: round(x * 2^7/ln2 + 127*128 - C).
EXP_A = 128.0 / float(np.log(2.0))
EXP_C = 7.7
EXP_B = 127.0 * 128.0 - EXP_C

KT_PIECE = 512  # kt DMA piece width (4 k-chunks)
VA_PIECE = 8  # va DMA piece size in k-chunks
ST_BUFS = 3
PT_BUFS = 4


def _build_program():
    nc = bacc.Bacc(
        "TRN2",
        target_bir_lowering=False,
        debug=False,
        num_devices=N_CORES,
    )
    qt_d = nc.declare_dram_parameter("QT", [D, LQ_SHARD], F16, isOutput=False)
    kt_d = nc.declare_dram_parameter("KT", [D, LK], F16, isOutput=False)
    va_d = nc.declare_dram_parameter("VA", [LK, D + 1], BF16, isOutput=False)
    ot_d = nc.declare_dram_parameter("OT", [D, LQ_SHARD], F32, isOutput=True)

    with tile.TileContext(nc) as tc, ExitStack() as ctx:
        singles = ctx.enter_context(tc.tile_pool(name="singles", bufs=1))
        st_pool = ctx.enter_context(
            tc.tile_pool(name="st", bufs=ST_BUFS, space="PSUM")
        )
        ot_pool = ctx.enter_context(tc.tile_pool(name="ot", bufs=1, space="PSUM"))
        pt_pool = ctx.enter_context(tc.tile_pool(name="pt", bufs=PT_BUFS))
        osb_pool = ctx.enter_context(tc.tile_pool(name="osb", bufs=2))
        out_pool = ctx.enter_context(tc.tile_pool(name="out", bufs=4))
        norm_pool = ctx.enter_context(tc.tile_pool(name="norm", bufs=8))

        # Preload the exp activation table while input DMAs run.
        warm = singles.tile([1, 2], F32)
        nc.vector.memset(warm[:, :], 0.0)
        nc.scalar.activation(
            out=warm[:, :], in_=warm[:, :],
            func=mybir.ActivationFunctionType.Exp,
        )

        qt_sb = singles.tile([D, LQ_SHARD], F16, name="qt")
        kt_sb = singles.tile([D, LK], F16, name="kt")
        va_sb = singles.tile([KC, NKC, D + 1], BF16, name="va")
        va_r = va_d[:, :].rearrange("(c p) d -> p c d", p=KC)

        # Input DMAs, issued in consumption order (kt chunks gate the
        # score matmuls; va chunks the PV matmuls; qt half 1 only needed
        # at the second q-block).
        def dma_kt(j):
            sl = slice(j * KT_PIECE, (j + 1) * KT_PIECE)
            nc.sync.dma_start(out=kt_sb[:, sl], in_=kt_d[:, sl])

        def dma_qt(h):
            sl = slice(h * QB, (h + 1) * QB)
            nc.sync.dma_start(out=qt_sb[:, sl], in_=qt_d[:, sl])

        def dma_va(h):
            sl = slice(h * VA_PIECE, (h + 1) * VA_PIECE)
            nc.sync.dma_start(out=va_sb[:, sl, :], in_=va_r[:, sl, :])

        dma_kt(0)
        dma_qt(0)
        dma_va(0)
        dma_kt(1)
        dma_kt(2)
        dma_va(1)
        dma_kt(3)
        dma_kt(4)
        dma_qt(1)
        dma_va(2)
        dma_kt(5)
        dma_kt(6)
        dma_va(3)
        dma_kt(7)

        def emit_scores(qb, c, st_ps):
            for s in range(NSL):
                sl = slice(s * SL, (s + 1) * SL)
                qsl = slice(qb * QB + s * SL, qb * QB + (s + 1) * SL)
                nc.tensor.matmul(
                    out=st_ps[:, sl],
                    lhsT=kt_sb[:, c * KC : (c + 1) * KC],
                    rhs=qt_sb[:, qsl],
                    start=True,
                    stop=True,
                )

        def emit_exp(c, st_ps, pt):
            # pt is an int16 tile; both writers produce bf16 bit patterns.
            if c % 2 == 0:
                nc.scalar.activation(
                    out=pt[:, :].bitcast(BF16),
                    in_=st_ps[:, :],
                    func=mybir.ActivationFunctionType.Exp,
                )
            else:
                nc.vector.tensor_scalar(
                    out=pt[:, :],
                    in0=st_ps[:, :],
                    scalar1=EXP_A,
                    scalar2=EXP_B,
                    op0=mybir.AluOpType.mult,
                    op1=mybir.AluOpType.add,
                )

        def emit_pv(c, pt, ot_ps):
            for s in range(NSL):
                sl = slice(s * SL, (s + 1) * SL)
                nc.tensor.matmul(
                    out=ot_ps[:, sl],
                    lhsT=va_sb[:, c, :],
                    rhs=pt[:, sl].bitcast(BF16),
                    start=(c == 0),
                    stop=(c == NKC - 1),
                )

        for qb in range(NQB):
            ot_ps = ot_pool.tile([D + 1, QB], F32)
            st_tiles = {}
            for c in range(ST_BUFS):  # prologue: scores run ahead
                st_tiles[c] = st_pool.tile([KC, QB], F32, tag="st", name="st")
                emit_scores(qb, c, st_tiles[c])
            for c in range(NKC):
                if c + ST_BUFS < NKC:
                    st_tiles[c + ST_BUFS] = st_pool.tile(
                        [KC, QB], F32, tag="st", name="st"
                    )
                    emit_scores(qb, c + ST_BUFS, st_tiles[c + ST_BUFS])
                pt = pt_pool.tile([KC, QB], I16, tag="pt", name="pt")
                emit_exp(c, st_tiles.pop(c), pt)
                emit_pv(c, pt, ot_ps)

            # Normalize O[d, q] = OT[d, q] / OT[64, q], in halves so the
            # recip/broadcast/mul/DMA chain pipelines across engines.
            if qb < NQB - 1:
                # Free the OT PSUM banks quickly for the next q-block.
                osb = osb_pool.tile([D + 1, QB], F32)
                nc.vector.tensor_copy(osb[:, :], ot_ps[:, :])
                src = osb
            else:
                src = ot_ps
            for h in range(2):
                sl = slice(h * SL, (h + 1) * SL)
                den = norm_pool.tile([1, SL], F32)
                nc.vector.tensor_copy(den[:, :], src[D : D + 1, sl])
                recip = norm_pool.tile([1, SL], F32)
                nc.vector.reciprocal_approx_fast(recip[:, :], den[:, :])
                bcast = norm_pool.tile([D, SL], F32)
                nc.gpsimd.partition_broadcast(bcast[:, :], recip[:, :])
                o_sb = out_pool.tile([D, SL], F32)
                nc.vector.tensor_mul(o_sb[:, :], src[0:D, sl], bcast[:, :])
                osl = slice(qb * QB + h * SL, qb * QB + (h + 1) * SL)
                nc.sync.dma_start(out=ot_d[:, osl], in_=o_sb[:, :])

    nc.finalize()
    return nc


_PROGRAM_CACHE = {}


def _get_program():
    if "nc" not in _PROGRAM_CACHE:
        _PROGRAM_CACHE["nc"] = _build_program()
    return _PROGRAM_CACHE["nc"]


def _make_in_maps(Q, K, V):
    Q = np.asarray(Q, dtype=np.float32)
    K = np.asarray(K, dtype=np.float32)
    V = np.asarray(V, dtype=np.float32)
    in_maps = []
    ones = np.ones((LK, 1), dtype=np.float32)
    for core in range(N_CORES):
        b, half = core // 2, core % 2
        q_shard = Q[b, half * LQ_SHARD : (half + 1) * LQ_SHARD, :]  # [2048, 64]
        qt = np.ascontiguousarray(q_shard.T).astype(np.float16)  # [64, 2048]
        kt = np.ascontiguousarray(K[b].T).astype(np.float16)  # [64, 4096]
        va = np.concatenate([V[b], ones], axis=1).astype(BF16NP)  # [4096, 65]
        in_maps.append({"QT": qt, "KT": kt, "VA": np.ascontiguousarray(va)})
    return in_maps


def _run(Q, K, V, trace=False, **spmd_kwargs):
    nc = _get_program()
    in_maps = _make_in_maps(Q, K, V)
    res = run_bass_kernel_spmd(
        nc, in_maps, list(range(N_CORES)), trace=trace, **spmd_kwargs
    )
    out = np.empty((B, LQ, D), dtype=np.float32)
    for core in range(N_CORES):
        b, half = core // 2, core % 2
        ot = res.results[core]["OT"]  # [64, 2048]
        out[b, half * LQ_SHARD : (half + 1) * LQ_SHARD, :] = ot.T
    return out, res


def kernel(Q, K, V):
    out, _ = _run(Q, K, V, trace=False)
    return out
